# revision 1
# baseline (speedup 1.0000x reference)
"""Conv2d 3x3 (pad 1, stride 1) + bias on 8 Trainium2 cores.

Problem: x [32,128,56,56] f32, weights [256,128,3,3] f32, bias [256] f32
         -> out [32,256,56,56] f32.

Strategy
--------
Data-parallel over batch: each of the 8 cores owns 4 images.

Per core, implicit GEMM on a shared-padding row layout with stride 57:
  - Padded pixel (ih, iw), ih in [-1,56], iw in [-1,56], lives at flat
    index (ih+1)*57 + (iw+1); the right pad of row r IS the left pad of
    row r+1 (one shared zero column), so the buffer is 58*57+1 = 3307
    floats per channel (+1 trailing zero -> 3308). Output is computed in
    the same stride-57 layout: every tap (kh, kw) of the 3x3 kernel is a
    CONSTANT offset kh*57+kw into the flat padded input, so one matmul
    covers 8 output rows at once (N = 8*57 = 456 <= 512 PSUM bank).
  - bf16 operands (tolerance is 2e-2; bf16 conv lands ~2e-3): same
    1 cycle/row PE rate as fp32r, half the input DMA bytes, and -- the
    point -- bf16 matmuls may be split into LDWEIGHTS + non-self-loading
    MATMUL, which fp32r cannot (hw bug). We exploit that split to DEDUP
    weight loads: taps are the outer loop over a group of output tiles,
    so consecutive matmuls share the same stationary tile and all but
    the first carry ldweights=False (no per-matmul reload).
  - 9 taps accumulate into each tile's PSUM bank (start/stop flags);
    Cout=256 is split into 2 halves of 128 partitions. Groups of 4/3
    tiles (7 PSUM banks + 1 warmup bank) double-buffer against drains.
  - Bias is fused into the PSUM->SBUF copy via DVE tensor_scalar_add.
  - Startup: the ~6.5us framework preamble gates everything; while the
    first chunks transfer, throwaway warmup matmuls ramp the PE clock.
    Image 0 half 0 is a tap-pass-split block (tile-outer taps 0-2,
    then tap-outer 3-8) whose consumption order matches DMA arrival
    order, so the startup transient causes no PE stalls (a stall also
    re-throttles the clock ramp). Image prefetches for b>=1 are gated
    behind the previous image's first drain by an artificial WAW dep:
    DMA transfers share the 16 engines' aggregate bandwidth, so an
    early 847KB prefetch would starve the chunks that gate the PE.
  - Tail: the last image-half ends in narrow 342/114-col tile groups
    whose output DMAs ride different queues (SP/ACT) so their
    descriptor generations overlap; after the final matmul only a
    114-col bias-add + a tiny DMA + the fixed completion semaphore
    (~0.9us) + end barrier remain.
  - Host strips the junk column per row at the end.

Built on bacc.Bacc: compile() runs move_matmul_waits_to_ldweights /
generate_event_semaphores passes that split bf16 matmuls into
LDWEIGHTS+MATMUL pairs and spill excess waits into EventSemaphores.
"""

import numpy as np
import ml_dtypes

import concourse.bacc as bacc
import concourse.mybir as mybir
import concourse.tile as tile
from concourse.bass_utils import run_bass_kernel_spmd

B, CIN, H, W = 32, 128, 56, 56
COUT = 256
NCORES = 8
BLOC = B // NCORES  # images per core
SP = W + 1  # 57: row stride of the shared-padding layout
NPIX = (H + 2) * SP + 2  # 3308 padded cols per channel
OUTW = H * SP  # 3192 output cols in stride-57 layout (1/57 junk)
TILE_N = 8 * SP  # 456: 8 output rows per PSUM tile
NTILES = 7  # 7 x 456 = 3192
NWARM = 4  # PE clock-ramp matmuls before the first data arrives

# Tile groups per (image, half) as (col_lo, ncols) runs: taps loop
# outermost within a group so the stationary weights are loaded once
# per tap per group. 4+3 tiles keeps <=7 PSUM banks in rotation (+1
# for warmup).
T = TILE_N
GROUPS = [
    [(0, T), (T, T), (2 * T, T), (3 * T, T)],
    [(4 * T, T), (5 * T, T), (6 * T, T)],
]
# Image 0 half 0 runs all 7 tiles as ONE block, split into a tap 0-2
# pass and a tap 3-8 pass. The first pass starts as soon as x chunk 1
# and the first weight chunk land and consumes tiles left-to-right at
# the rate the x chunks arrive; by the time the second pass needs tap
# 3+ weights (~4us later), their chunk has drained through the shared
# DMA engines. This absorbs the startup DMA transient with zero PE
# stalls (a stall would also re-throttle the PE clock ramp).
XCHUNKS0 = [(0, 288), (576, 1032), (1032, 1944), (1944, 2856), (2856, NPIX)]
# [288,576) rides the ACT queue so x chunk 1 lands via two queues at once
# The very last image-half ends in a lone HALF-width tile so the
# final drain + output DMA chain after the last matmul is as short as
# possible (the 0.7us DMA generation + 0.9us completion-semaphore
# latency are fixed; only the add and transfer scale with width).
GROUPS_LAST = [
    [(0, T), (T, T), (2 * T, T), (3 * T, T)],
    [(4 * T, T), (5 * T, T)],
    [(6 * T, 342)],
    [(6 * T + 342, 114)],
]

_nc_cache = None


def _build():
    f32 = mybir.dt.float32
    bf16 = mybir.dt.bfloat16
    nc = bacc.Bacc("TRN2", target_bir_lowering=False)
    x_d = nc.dram_tensor("xp", [BLOC, CIN, NPIX], bf16, kind="ExternalInput")
    w_d = nc.dram_tensor("wT", [CIN, 9 * COUT], bf16, kind="ExternalInput")
    b_d = nc.dram_tensor("bias2", [128, 2], f32, kind="ExternalInput")
    o_d = nc.dram_tensor("out", [BLOC, COUT, OUTW], f32, kind="ExternalOutput")

    with tile.TileContext(nc) as tc:
        with (
            tc.tile_pool(name="wpool", bufs=1) as wpool,
            tc.tile_pool(name="xpool", bufs=2) as xpool,
            tc.tile_pool(name="opool", bufs=4) as opool,
            tc.tile_pool(name="psum", bufs=7, space="PSUM") as psum,
            tc.tile_pool(name="wupsum", bufs=1, space="PSUM") as wupsum,
        ):
            # The 16 DMA engines are shared across queues, so big image
            # prefetches can starve the small weight chunks whose
            # completion gates the PE. Priority order on the ACT queue
            # (its ring drains in order): weight chunks sized to beat
            # each tap's first use, bias, THEN the image prefetches for
            # b>=1. The SP queue carries image 0's chunks + outputs.
            wsb = wpool.tile([CIN, 9 * COUT], bf16)
            bsb = wpool.tile([128, 2], f32)

            # PE warmup: bf16 throwaway matmuls on a memset tile bring
            # the HAM clock gate up while the first input chunks
            # transfer. The memset runs on GpSimd, whose queue clears
            # its preamble earliest, so the PE starts ramping ~1.5us
            # before the first data lands.
            wub = wpool.tile([128, 512], bf16)
            nc.vector.memset(wub[:], 0.0)

            xp0 = xpool.tile([CIN, NPIX], bf16, tag="xp", name="xp0")
            xp0g = xp0
            wup = wupsum.tile([128, 512], f32)
            nc.tensor.matmul(
                wup[:], lhsT=wub[:, :128], rhs=wub[:],
                start=True, stop=True,
            )
            # Gate the non-critical startup DMAs (taps 3-8 weights, half
            # 1 weights, bias -- none needed before ~12us) behind warmup
            # matmul 1 via a WAW dep: these throwaway 2-col writes into
            # the DMA target ranges read the warmup PSUM tile, so the
            # transfers stay out of the window where the chunks gating
            # the first real matmul need the DMA engines to themselves.
            nc.vector.tensor_scalar_mul(wsb[:, 384:386], wup[:, :2], 0.0)
            nc.vector.tensor_scalar_mul(wsb[:, 1152:1154], wup[:, :2], 0.0)
            nc.vector.tensor_scalar_mul(bsb[:, :2], wup[:, :2], 0.0)
            for gc in (1032, 1944, 2856):
                nc.vector.tensor_scalar_mul(
                    xp0g[:, gc : gc + 2], wup[:, :2], 0.0
                )
            for _ in range(NWARM - 1):
                nc.tensor.matmul(
                    wup[:], lhsT=wub[:, :128], rhs=wub[:],
                    start=True, stop=True,
                )

            # DMA transfers drain at the shared engines roughly in
            # generation order (~360GB/s aggregate, +0.9us completion
            # semaphore), so the chunks gating the first matmul (x cols
            # [0,576) split across both queues + taps 0-2 weights) are
            # generated first and kept small.
            xps = [xp0]
            nc.scalar.dma_start(xps[0][:, 288:576], x_d[0, :, 288:576])
            nc.scalar.dma_start(wsb[:, :384], w_d[:, :384])
            for lo, hi in XCHUNKS0:
                nc.sync.dma_start(xps[0][:, lo:hi], x_d[0, :, lo:hi])
            nc.scalar.dma_start(wsb[:, 384:1152], w_d[:, 384:1152])
            nc.scalar.dma_start(wsb[:, 1152:], w_d[:, 1152:])
            nc.scalar.dma_start(bsb[:], b_d[:])

            # --- image 0, half 0: tap-pass-split over all 7 tiles ---
            pts7 = [
                psum.tile([128, TILE_N], f32, tag="pt", name=f"pt7_{k}")
                for k in range(NTILES)
            ]
            # Pass A is TILE-outer (each tile's taps 0-2 together):
            # x chunks arrive left-to-right, so tile t's data must not
            # be needed before ~0.6us * t. Pass B is tap-outer for the
            # weight-load dedup; by then all chunks have landed.
            for t in range(NTILES):
                for tap in range(3):
                    kh, kw = divmod(tap, 3)
                    off = t * TILE_N + kh * SP + kw
                    nc.tensor.matmul(
                        pts7[t][:],
                        lhsT=wsb[:, tap * 128 : tap * 128 + 128],
                        rhs=xps[0][:, off : off + TILE_N],
                        start=(tap == 0),
                        stop=False,
                    )
            for tap in range(3, 9):
                kh, kw = divmod(tap, 3)
                for t in range(NTILES):
                    off = t * TILE_N + kh * SP + kw
                    nc.tensor.matmul(
                        pts7[t][:],
                        lhsT=wsb[:, tap * 128 : tap * 128 + 128],
                        rhs=xps[0][:, off : off + TILE_N],
                        start=False,
                        stop=(tap == 8),
                    )
            for lo, hi in ((0, 4), (4, 7)):
                ot0 = opool.tile([128, (hi - lo) * TILE_N], f32, tag="ot", name="ot0")
                for j, t in enumerate(range(lo, hi)):
                    nc.vector.tensor_scalar_add(
                        ot0[:, j * TILE_N : (j + 1) * TILE_N],
                        pts7[t][:],
                        bsb[:, 0:1],
                    )
                nc.sync.dma_start(
                    o_d[0, :128, lo * TILE_N : hi * TILE_N],
                    ot0[:, : (hi - lo) * TILE_N],
                )
                if lo == 0:
                    # Gate image 1's prefetch behind the first drain
                    # (see comment in the main loop below).
                    xp1 = xpool.tile([CIN, NPIX], bf16, tag="xp", name="xp1")
                    xps.append(xp1)
                    nc.gpsimd.tensor_scalar_mul(xp1[:, :2], ot0[:, :2], 0.0)
                    nc.sync.dma_start(xp1[:], x_d[1])

            for b in range(BLOC):
                xp = xps[b]
                for h in range(2):
                    if b == 0 and h == 0:
                        continue
                    if b == BLOC - 1 and h == 1:
                        groups = GROUPS_LAST
                    else:
                        groups = GROUPS
                    for gi, grp in enumerate(groups):
                        pts = [
                            psum.tile([128, n], f32, tag="pt", name=f"pt{k}")
                            for k, (lo, n) in enumerate(grp)
                        ]
                        for tap in range(9):
                            kh, kw = divmod(tap, 3)
                            c0 = h * (9 * 128) + tap * 128
                            for j, (lo, n) in enumerate(grp):
                                off = lo + kh * SP + kw
                                nc.tensor.matmul(
                                    pts[j][:],
                                    lhsT=wsb[:, c0 : c0 + 128],
                                    rhs=xp[:, off : off + n],
                                    start=(tap == 0),
                                    stop=(tap == 8),
                                )
                        w_out = sum(n for lo, n in grp)
                        c_lo = grp[0][0]
                        ot = opool.tile([128, w_out], f32, tag="ot")
                        col = 0
                        for j, (lo, n) in enumerate(grp):
                            nc.vector.tensor_scalar_add(
                                ot[:, col : col + n],
                                pts[j][:],
                                bsb[:, h : h + 1],
                            )
                            col += n
                        # The last two (half-width) groups of the whole
                        # kernel ship on the otherwise-idle ACT queue so
                        # their descriptor generation overlaps the SP
                        # queue's bulk output DMA.
                        oq = nc.scalar if groups is GROUPS_LAST and gi == 3 else nc.sync
                        oq.dma_start(
                            o_d[b, h * 128 : (h + 1) * 128, c_lo : c_lo + w_out],
                            ot[:, :w_out],
                        )
                        if h == 0 and gi == 0 and b + 1 < BLOC:
                            # Prefetch the next image, gated behind this
                            # image's first drain by a REAL dependency (a
                            # throwaway 2-col write the DMA then WAW-waits
                            # on): the 8 DMAHW queues share the 16 DMA
                            # engines' bandwidth, so an early 847KB
                            # prefetch starves the small weight/input
                            # chunks whose completion gates the PE. The
                            # Tile scheduler hoists dep-free DMAs past
                            # queue order, so position alone cannot hold
                            # the prefetch back.
                            xpn = xpool.tile([CIN, NPIX], bf16, tag="xp", name=f"xp{b+1}")
                            xps.append(xpn)
                            nc.gpsimd.tensor_scalar_mul(
                                xpn[:, :2], ot[:, :2], 0.0
                            )
                            nc.sync.dma_start(xpn[:], x_d[b + 1])

    nc.compile()
    n = _dedup_ldweights(nc)
    assert n >= 300, f"ldweights dedup removed only {n}"
    return nc


def _dedup_ldweights(nc):
    """Delete redundant InstLdweights after compile.

    compile()'s move_matmul_waits_to_ldweights splits every bf16 matmul
    into an InstLdweights + InstMatmult(ldweights=False) pair. Taps are
    the outer loop within each tile group, so consecutive pairs mostly
    reload the exact same stationary [128,128] tile; the PE array keeps
    its weights between matmuls, so a reload whose access pattern equals
    the previous load on the queue is dead. Only sync-free loads are
    dropped (waits/updates stay where the compiler put them).
    """
    removed = set()
    for blk in nc.main_func.blocks:
        keep = []
        cur = None
        for i in blk.instructions:
            if isinstance(i, mybir.InstLdweights):
                key = str(i.ins[0])
                si = i.sync_info
                sync_free = si is None or (not si.on_wait and not si.on_update)
                if key == cur and sync_free:
                    removed.add(i.name)
                    continue
                cur = key
            keep.append(i)
        blk.instructions[:] = keep
    if removed:
        for blk in nc.main_func.blocks:
            for i in blk.instructions:
                for n in removed:
                    try:
                        i.try_remove_dependency(n)
                    except Exception:
                        pass
    return len(removed)


def _get_nc():
    global _nc_cache
    if _nc_cache is None:
        _nc_cache = _build()
    return _nc_cache


def _prep_inputs(x, weights, bias):
    x = np.asarray(x, dtype=np.float32)
    weights = np.asarray(weights, dtype=np.float32)
    bias = np.ascontiguousarray(np.asarray(bias, dtype=np.float32))

    # Shared-padding stride-57 layout: rows -1..56 at stride 57 plus one
    # trailing zero (the last row's right pad) + one more pad col.
    xb = x.astype(ml_dtypes.bfloat16)
    xpad = np.pad(xb, ((0, 0), (0, 0), (1, 1), (1, 0))).reshape(B, CIN, (H + 2) * SP)
    xp = np.concatenate(
        [xpad, np.zeros((B, CIN, 2), dtype=ml_dtypes.bfloat16)], axis=2
    )  # [B, CIN, 3308]
    # [Cout,Cin,3,3] -> [Cin, (half kh kw co)] so each (half, tap) lhsT
    # slice is a contiguous [128,128] stationary tile.
    wT = np.ascontiguousarray(
        weights.reshape(2, 128, CIN, 3, 3).transpose(2, 0, 3, 4, 1)
    ).reshape(CIN, 9 * COUT).astype(ml_dtypes.bfloat16)
    b2 = np.ascontiguousarray(bias.reshape(2, 128).T)  # b2[p, h] = bias[h*128+p]

    return [
        {
            "xp": np.ascontiguousarray(xp[i * BLOC : (i + 1) * BLOC]),
            "wT": wT,
            "bias2": b2,
        }
        for i in range(NCORES)
    ]


def _run(inputs, trace=False):
    in_maps = _prep_inputs(inputs["x"], inputs["weights"], inputs["bias"])
    res = run_bass_kernel_spmd(
        _get_nc(), in_maps, core_ids=list(range(NCORES)), trace=trace
    )
    out = np.concatenate([r["out"] for r in res.results], axis=0)  # [B, COUT, 3192]
    out = out.reshape(B, COUT, H, SP)[:, :, :, :W]
    return np.ascontiguousarray(out), res


def kernel(x, weights, bias):
    out, _ = _run({"x": x, "weights": weights, "bias": bias})
    return out



# revision 12
# speedup vs baseline: 1.2955x; 1.2955x over previous
"""Conv2d 3x3 (pad 1, stride 1) + bias on 8 Trainium2 cores.

Problem: x [32,128,56,56] f32, weights [256,128,3,3] f32, bias [256] f32
         -> out [32,256,56,56] f32.

Strategy
--------
Data-parallel over batch (4 images/core) + 1D Winograd F(2,3) along W.

For each output pair (2u, 2u+1) and each vertical tap kh, the 3-tap
horizontal conv costs 4 multiplies instead of 6: with d0..d3 the 4
padded inputs around the pair,
  t0 = d0-d2, t1 = d1+d2, t2 = d2-d1, t3 = d1-d3
  m_p = sum_cin w'_p * t_p   (w'_0=g0, w'_1=(g0+g1+g2)/2,
                              w'_2=(g0-g1+g2)/2, w'_3=g2)
  out[2u]   = m0+m1+m2+bias
  out[2u+1] = m1-m2-m3+bias
PE work drops from 9 to 6 matmul-columns per output pixel (the junk
57-stride column of the direct kernel also disappears): 150.5K cols/core
= 62.7us at 2.4GHz vs 95.8us direct.

Layout: host splits the padded rows into even/odd column planes
(xe/xo, [58 rows, 29 cols]) so the four t-plane transforms are
CONTIGUOUS tensor_tensor ops on DVE (2-byte packed -> 2x mode). The
t-planes [cin, 58*28] use flat row-stride 28; vertical tap kh of a
group at flat col lo is the constant offset lo + kh*28, so matmuls of
up to 512 cols run seamlessly across row boundaries (same trick as the
direct kernel's 57-stride).

Per group of N=392 pair-cols: 12 matmuls (4 m-banks x 3 kh) accumulate
in 4 PSUM banks; 2 groups double-buffer across the 8 banks. The A^T
combine is split across three engines so none becomes the bottleneck:
  DVE:    u0 = (m0+bias)+m1        (scalar_tensor_tensor, psum)
  GpSimd: u1 = (m1+bias)-m2        (scalar_tensor_tensor, psum)
  ACT:    v2 = copy(m2), v3 = copy(-m3)  (activation, psum->bf16)
  DVE:    out0 = u0+v2, out1 = u1+v3    (bf16 packed, 2x mode)
Outputs stay as separate even/odd planes in bf16; the host interleaves
and widens to f32 (tolerance is 2e-2; bf16 out lands ~4e-3).

Startup mirrors the direct kernel's proven choreography: warmup
matmuls ramp the PE clock while the first DMA wave (bias, first
weights, first xe/xo row-chunks) lands; image 0 half 0 starts with
small taper groups (112/280 cols) so the first matmuls need only 7
input rows; non-critical transfers (later chunks, half-1 weights,
image prefetches) are gated behind warmup/first-drain WAW touches so
they cannot starve the critical first wave. Transforms for image b+1
run on DVE interleaved with image b's half-1 drains. The final half
tapers (392,392,392,280,112) and ships its last two output planes on
separate queues.
"""

import numpy as np
import ml_dtypes

import concourse.bacc as bacc
import concourse.mybir as mybir
import concourse.tile as tile
from concourse.bass_utils import run_bass_kernel_spmd
from concourse.alu_op_type import AluOpType

B, CIN, H, W = 32, 128, 56, 56
COUT = 256
NCORES = 8
BLOC = B // NCORES  # images per core
NR = H + 2  # 58 padded rows
PW = W // 2 + 1  # 29 even/odd plane cols
PC = W // 2  # 28 output pairs per row
PLANE = NR * PC  # 1624 flat t-plane cols
NPAIR = H * PC  # 1568 output pair-cols per image-half
NWARM = 4

# Weight stationary order per half = first-use order: m1, m2, m0, m3.
WORDER = [1, 2, 0, 3]
PIDX = {p: i for i, p in enumerate(WORDER)}

NORM_GROUPS = [(0, 392), (392, 392), (784, 392), (1176, 392)]
# Image 0 half 0: start-taper so the first matmuls need only 7 input
# rows (the first DMA chunk); rows needed per group stay behind the
# chunked transforms tc0..tc4.
FIRST_GROUPS = [(0, 112), (112, 280), (392, 392), (784, 392), (1176, 392)]
# Last half: end-taper so the final drain + output DMA chain after the
# last matmul is as short as possible.
LAST_GROUPS = [(0, 392), (392, 392), (784, 392), (1176, 280), (1456, 112)]
# Image 0 xe/xo row chunks (DMA + transform granularity).
CHUNKS0 = [(0, 7), (7, 17), (17, 31), (31, 45), (45, 58)]

_nc_cache = None


def _build():
    f32 = mybir.dt.float32
    bf16 = mybir.dt.bfloat16
    COPY = mybir.ActivationFunctionType.Copy
    IDENT = mybir.ActivationFunctionType.Identity
    nc = bacc.Bacc("TRN2", target_bir_lowering=False)
    x_d = nc.dram_tensor("xeo", [BLOC, CIN, 2, NR, PW], bf16, kind="ExternalInput")
    w_d = nc.dram_tensor("wT", [CIN, 2 * 4 * 3 * 128], bf16, kind="ExternalInput")
    b_d = nc.dram_tensor("bias2", [128, 2], f32, kind="ExternalInput")
    o_d = nc.dram_tensor("out", [BLOC, 2, 128, 2, NPAIR], bf16, kind="ExternalOutput")

    def wcol(h, p, kh):
        return ((h * 4 + PIDX[p]) * 3 + kh) * 128

    with tile.TileContext(nc) as tc:
        with (
            tc.tile_pool(name="wpool", bufs=1) as wpool,
            tc.tile_pool(name="xpool", bufs=2) as xpool,
            tc.tile_pool(name="tpool", bufs=8) as tpool,
            tc.tile_pool(name="upool", bufs=3) as upool,
            tc.tile_pool(name="vpool", bufs=3) as vpool,
            tc.tile_pool(name="opool", bufs=4) as opool,
            tc.tile_pool(name="psum", bufs=8, space="PSUM") as psum,
        ):
            wsb = wpool.tile([CIN, 2 * 4 * 3 * 128], bf16)
            bsb = wpool.tile([128, 2], f32)
            wub = wpool.tile([128, 512], bf16)
            dmy = wpool.tile([128, 2], bf16)
            nc.gpsimd.memset(wub[:], 0.0)
            # Dummy Identity activation: pulls the ~1.3us activation
            # table load to the front of the ACT queue (its engine queue
            # depth is 0, so a late table load would stall the queue).
            nc.scalar.activation(dmy[:], wub[:, :2], IDENT)

            # xe/xo for images, double-buffered; planes per image.
            xeos = [xpool.tile([CIN, 2, NR, PW], bf16, tag="xeo", name="xeo0")]
            tpls = [
                [
                    tpool.tile([CIN, NR, PC], bf16, tag="tp", name=f"tp0_{p}")
                    for p in range(4)
                ]
            ]

            # PE warmup: ramp the clock while the first DMA wave lands.
            wup = psum.tile([128, 512], f32, tag="pt", name="wup")
            nc.tensor.matmul(
                wup[:], lhsT=wub[:, :128], rhs=wub[:], start=True, stop=True
            )
            # Gate non-critical startup DMAs behind warmup matmul 1 via
            # WAW touches (the DMA then waits on the 2-col write).
            nc.vector.tensor_scalar_mul(wsb[:, 1536:1538], wup[:, :2], 0.0)
            xeo0 = xeos[0]
            for (r0, r1) in CHUNKS0[2:]:
                nc.vector.tensor_scalar_mul(
                    xeo0[:, 0, r0, 0:2], wup[:, :2], 0.0
                )
            for _ in range(NWARM - 1):
                nc.tensor.matmul(
                    wup[:], lhsT=wub[:, :128], rhs=wub[:], start=True, stop=True
                )

            # Startup DMA wave, all on the SP queue in arrival-deadline
            # order (the queue's ring drains in order): bias, chunk 0,
            # half-0 weights, chunk 1, then the gated later chunks.
            # Half-1 weights ride the otherwise-idle ACT queue (gated).
            c0, c1 = CHUNKS0[0], CHUNKS0[1]
            nc.sync.dma_start(bsb[:], b_d[:])
            nc.sync.dma_start(
                xeo0[:, :, c0[0] : c0[1], :], x_d[0, :, :, c0[0] : c0[1], :]
            )
            nc.sync.dma_start(wsb[:, 0:1536], w_d[:, 0:1536])
            nc.sync.dma_start(
                xeo0[:, :, c1[0] : c1[1], :], x_d[0, :, :, c1[0] : c1[1], :]
            )
            for (r0, r1) in CHUNKS0[2:]:
                nc.sync.dma_start(
                    xeo0[:, :, r0:r1, :], x_d[0, :, :, r0:r1, :]
                )
            nc.scalar.dma_start(wsb[:, 1536:], w_d[:, 1536:])

            def transform(bi, r0, r1, only_p=None):
                """t-plane rows [r0,r1) for image slot bi (DVE)."""
                xeo = xeos[bi]
                xe = xeo[:, 0]
                xo = xeo[:, 1]
                tp = tpls[bi]
                ops = {
                    0: (nc.vector.tensor_sub, xe[:, r0:r1, 0:PC], xe[:, r0:r1, 1 : PC + 1]),
                    1: (nc.vector.tensor_add, xo[:, r0:r1, 0:PC], xe[:, r0:r1, 1 : PC + 1]),
                    2: (nc.vector.tensor_sub, xe[:, r0:r1, 1 : PC + 1], xo[:, r0:r1, 0:PC]),
                    3: (nc.vector.tensor_sub, xo[:, r0:r1, 0:PC], xo[:, r0:r1, 1 : PC + 1]),
                }
                order = [only_p] if only_p is not None else WORDER
                for p in order:
                    fn, a, b_ = ops[p]
                    fn(tp[p][:, r0:r1, :], a, b_)

            transform(0, *CHUNKS0[0])
            transform(0, *CHUNKS0[1])

            def do_group(b, bi, h, lo, n, last_half=False, last_group=False,
                         after_drains=None, prefetch=False):
                tp = tpls[bi]
                flat = [tp[p][:].rearrange("c r u -> c (r u)") for p in range(4)]
                pts = {}
                for p in WORDER:
                    pts[p] = psum.tile([128, 392], f32, tag="pt", name=f"pt_b{b}h{h}l{lo}p{p}")
                    for kh in range(3):
                        c = wcol(h, p, kh)
                        nc.tensor.matmul(
                            pts[p][:, :n],
                            lhsT=wsb[:, c : c + 128],
                            rhs=flat[p][:, lo + kh * PC : lo + kh * PC + n],
                            start=(kh == 0),
                            stop=(kh == 2),
                        )
                a1 = vpool.tile([128, 392], bf16, tag="a1")
                a2 = vpool.tile([128, 392], bf16, tag="a2")
                a3 = vpool.tile([128, 392], bf16, tag="a3")
                u0 = upool.tile([128, 392], bf16, tag="u0")
                w1 = upool.tile([128, 392], bf16, tag="w1")
                ot = opool.tile([128, 2, 392], bf16, tag="ot")
                bvec = bsb[:, h : h + 1]
                # PSUM reads: GpSimd can't touch PSUM and two-tensor
                # DVE ops may read at most one PSUM operand, so ACT
                # drains m1 (+bias), m2, -m3 to SBUF bf16 and DVE's
                # only PSUM op is m0 + a1.
                nc.scalar.activation(a1[:, :n], pts[1][:, :n], IDENT, bias=bvec)
                nc.scalar.activation(a2[:, :n], pts[2][:, :n], COPY)
                nc.scalar.activation(a3[:, :n], pts[3][:, :n], COPY, scale=-1.0)
                # out0 = (m0 + a1) + a2 ; out1 = (a1 - a2) + a3
                nc.vector.tensor_add(u0[:, :n], pts[0][:, :n], a1[:, :n])
                nc.vector.tensor_add(ot[:, 0, :n], u0[:, :n], a2[:, :n])
                nc.vector.tensor_sub(w1[:, :n], a1[:, :n], a2[:, :n])
                nc.gpsimd.tensor_add(ot[:, 1, :n], w1[:, :n], a3[:, :n])
                if last_group:
                    # Ship the two final planes on different queues so
                    # their descriptor generations overlap.
                    nc.sync.dma_start(o_d[b, h, :, 0, lo : lo + n], ot[:, 0, :n])
                    nc.scalar.dma_start(o_d[b, h, :, 1, lo : lo + n], ot[:, 1, :n])
                else:
                    nc.sync.dma_start(o_d[b, h, :, :, lo : lo + n], ot[:, :, :n])
                if prefetch:
                    # Prefetch next image's xe/xo, gated behind this
                    # group's first output (the 8 DMAHW queues share the
                    # 16 engines' bandwidth; an early 861KB prefetch
                    # would starve the transfers gating the PE).
                    xeon = xpool.tile([CIN, 2, NR, PW], bf16, tag="xeo", name=f"xeo{b+1}")
                    xeos.append(xeon)
                    tpls.append(
                        [
                            tpool.tile([CIN, NR, PC], bf16, tag="tp", name=f"tp{b+1}_{p}")
                            for p in range(4)
                        ]
                    )
                    nc.gpsimd.tensor_scalar_mul(
                        xeon[:, 0, 0, 0:2], ot[:, 0, 0:2], 0.0
                    )
                    nc.sync.dma_start(xeon[:], x_d[b + 1])
                if after_drains is not None:
                    after_drains()

            for b in range(BLOC):
                bi = b
                for h in range(2):
                    if b == 0 and h == 0:
                        groups = FIRST_GROUPS
                    elif b == BLOC - 1 and h == 1:
                        groups = LAST_GROUPS
                    else:
                        groups = NORM_GROUPS
                    for gi, (lo, n) in enumerate(groups):
                        after = None
                        if b == 0 and h == 0 and gi < 3:
                            r0, r1 = CHUNKS0[gi + 2]
                            after = lambda r0=r0, r1=r1: transform(0, r0, r1)
                        elif h == 1 and b + 1 < BLOC and gi < 4:
                            p = [1, 0, 2, 3][gi]
                            after = lambda b=b, p=p: transform(b + 1, 0, NR, only_p=p)
                        do_group(
                            b, bi, h, lo, n,
                            last_group=(b == BLOC - 1 and h == 1 and gi == len(groups) - 1),
                            after_drains=after,
                            prefetch=(h == 0 and gi == 0 and b + 1 < BLOC),
                        )

    nc.compile()
    return nc


def _get_nc():
    global _nc_cache
    if _nc_cache is None:
        _nc_cache = _build()
    return _nc_cache


def _prep_inputs(x, weights, bias):
    x = np.asarray(x, dtype=np.float32)
    weights = np.asarray(weights, dtype=np.float32)
    bias = np.ascontiguousarray(np.asarray(bias, dtype=np.float32))

    xb = x.astype(ml_dtypes.bfloat16)
    xpad = np.pad(xb, ((0, 0), (0, 0), (1, 1), (1, 1)))  # [B,C,58,58]
    xe = xpad[:, :, :, 0::2]  # [B,C,58,29]
    xo = xpad[:, :, :, 1::2]
    xeo = np.ascontiguousarray(np.stack([xe, xo], axis=2))  # [B,C,2,58,29]

    g = weights.reshape(2, 128, CIN, 3, 3)  # [h, co, cin, kh, kw]
    w1 = (g[..., 0] + g[..., 1] + g[..., 2]) * 0.5
    w0 = g[..., 0]
    w2 = (g[..., 0] - g[..., 1] + g[..., 2]) * 0.5
    w3 = g[..., 2]
    # stack in WORDER; axes [h, p, co, cin, kh]
    wlist = [w0, w1, w2, w3]
    wp = np.stack([wlist[p] for p in WORDER], axis=1)
    # -> [cin, h, p, kh, co] -> [128, 3072]
    wT = np.ascontiguousarray(wp.transpose(3, 0, 1, 4, 2)).reshape(
        CIN, 2 * 4 * 3 * 128
    ).astype(ml_dtypes.bfloat16)
    b2 = np.ascontiguousarray(bias.reshape(2, 128).T)  # b2[p,h] = bias[h*128+p]

    return [
        {
            "xeo": np.ascontiguousarray(xeo[i * BLOC : (i + 1) * BLOC]),
            "wT": wT,
            "bias2": b2,
        }
        for i in range(NCORES)
    ]


def _run(inputs, trace=False):
    in_maps = _prep_inputs(inputs["x"], inputs["weights"], inputs["bias"])
    res = run_bass_kernel_spmd(
        _get_nc(), in_maps, core_ids=list(range(NCORES)), trace=trace
    )
    o = np.concatenate([np.asarray(r["out"]) for r in res.results], axis=0)
    # [B, 2h, 128co, 2pl, 1568] bf16 -> [B, 256, 56, 56] f32
    o = o.astype(np.float32).reshape(B, 2, 128, 2, H, PC)
    o = o.transpose(0, 1, 2, 4, 5, 3).reshape(B, COUT, H, W)
    return np.ascontiguousarray(o), res


def kernel(x, weights, bias):
    out, _ = _run({"x": x, "weights": weights, "bias": bias})
    return out
